# revision 30
# baseline (speedup 1.0000x reference)
"""Causal multi-head self-attention (B=4, S=2048, D=1024, H=16, RoPE) on 8
Trainium2 NeuronCores.

Sharding (hardcoded): core c handles batch b = c//2 and head group g = c%2
(8 of the 16 heads).  Data parallel over B, tensor parallel over heads for
the Wq/Wk/Wv projections and over Wo rows/columns: each core computes the
attention output for its 8 heads, the two cores of a pair AllGather their
(normalized) per-head-pair attention chunks once per (hp, sq-tile), and
each core then computes a disjoint 512-wide column slice of the final Wo
projection for its batch, so the host only concatenates slices.

Kernel structure (tensor-engine-bound design):
  * sq-tile (j) OUTER loop in order [0, 1, 3, 2], head-pair (hp) inner.
    Attention runs flash-style over transposed [sk 128, sq 512] score
    blocks with the QK -> exp -> PV chain software-pipelined two blocks
    deep.  Projections for all remaining (hp, j) tiles sit in a global
    generator queue, drip-fed between attention blocks as tensor-engine
    filler at a fixed aggressive rate (front-loading ramps the PE clock);
    a deadline drain force-completes the next tile's projections one
    segment early so dependencies are always ready.
  * Diagonal score blocks are column-narrowed and the causal triangle of
    the leading 128-strip is folded into the QK PSUM accumulation as a
    rank-128 bias matmul, so exp yields ~0 there with no mask op.
  * Denominators ride as extra lhsT columns through the PV matmul (PSUM
    rows 64 and 65 for the two heads), so one 2-lane DVE
    reciprocal_approx_fast straight out of PSUM plus a raw bf16 copy of
    rows 0..63 frees the PV banks in ~2us; the gpsimd partition
    broadcasts and the normalization multiplies trail off the critical
    path.
  * Rope evacuates projection PSUM without a scalar-engine copy: the DVE
    cos-multiply and stream_shuffle read PSUM directly (freeing the bank
    after two ops), the sin-multiply runs on gpsimd.
  * Wo accumulates hp-major (matching AllGather arrival order) so only
    the last pair of matmuls in each 8-matmul unit depends on the final
    gather of a tile.  Tiles run in order 0, 1, 3, 2 and the Wo work for
    tiles 3 and 2 is held back to the epilogue, where it covers the last
    AllGather's ~13us latency with ~13us of ready matmuls.
  * PSUM budget (8 banks): 2x2 double-buffered QK scores, 2 PV
    accumulator, 2 rotating for projections/Wo/transposes.
"""

import numpy as np

D_MODEL = 1024
NUM_HEADS = 16
ROPE_THETA = 10000.0
DH = D_MODEL // NUM_HEADS  # 64
SQT = 512  # sq tile width (= PSUM bank width in f32)


# ---------------------------------------------------------------------------
# Device kernel builder
# ---------------------------------------------------------------------------

def build_kernel(n_cores: int = 8, S: int = 2048):
    import concourse.bass as bass
    import concourse.mybir as mybir
    import concourse.tile as tile
    from concourse import bacc
    from concourse.masks import make_identity

    F32 = mybir.dt.float32
    BF16 = mybir.dt.bfloat16
    Exp = mybir.ActivationFunctionType.Exp
    mult = mybir.AluOpType.mult
    add = mybir.AluOpType.add

    D = D_MODEL
    NC = D // 128          # 8 d-chunks
    NSB = S // 128         # s 128-blocks
    NSQ = S // SQT         # sq 512-tiles
    NHP = 4                # head pairs per core
    SWAP16 = list(range(16, 32)) + list(range(16))
    TILE_ORDER = [0, 1, 3, 2]
    ORD = {j: i for i, j in enumerate(TILE_ORDER)}

    nc = bacc.Bacc("TRN2", target_bir_lowering=False, debug=False,
                   num_devices=n_cores)

    xT = nc.dram_tensor("xT", [128, NC, S], BF16, kind="ExternalInput")
    wqT = nc.dram_tensor("wqT", [128, NHP, NC, 128], BF16, kind="ExternalInput")
    wkT = nc.dram_tensor("wkT", [128, NHP, NC, 128], BF16, kind="ExternalInput")
    wvT = nc.dram_tensor("wvT", [128, NHP, NC, 128], BF16, kind="ExternalInput")
    woT = nc.dram_tensor("woT", [128, NC, SQT], BF16, kind="ExternalInput")
    cosT = nc.dram_tensor("cosT", [128, S], BF16, kind="ExternalInput")
    sinT = nc.dram_tensor("sinT", [128, S], BF16, kind="ExternalInput")
    # triangle-bias matmul constants: biasL.T @ biasR = -B on the
    # strictly-lower-triangular (masked) region of a 128x128 block
    biasLT = nc.dram_tensor("biasLT", [128, 128], BF16, kind="ExternalInput")
    biasRT = nc.dram_tensor("biasRT", [128, 128], BF16, kind="ExternalInput")
    out = nc.dram_tensor("out", [S, SQT], F32, kind="ExternalOutput")

    groups = [[2 * i, 2 * i + 1] for i in range(n_cores // 2)]

    with tile.TileContext(nc) as tc:
        with (
            tc.tile_pool(name="const", bufs=1) as constp,
            tc.tile_pool(name="vt", bufs=2) as vtp,
            tc.tile_pool(name="probs", bufs=4) as probsp,
            tc.tile_pool(name="rope", bufs=3) as ropep,
            tc.tile_pool(name="attn", bufs=2) as attnp,
            tc.tile_pool(name="ag", bufs=2) as agp,
            tc.tile_pool(name="norm", bufs=2) as normp,
            tc.tile_pool(name="ost", bufs=2) as ostp,
            tc.tile_pool(name="psQK", bufs=2, space="PSUM") as psQK,
            tc.tile_pool(name="psPV", bufs=1, space="PSUM") as psPV,
            tc.tile_pool(name="psM", bufs=2, space="PSUM") as psM,
            tc.tile_pool(name="dram", bufs=2, space="DRAM") as dramp,
        ):
            # --- one-time loads (order = sync-ring order: small weights
            # and tables first so the first projections can start early;
            # x streams in per-chunk behind them; wo last) ----------------
            wq_sb = constp.tile([128, NHP, NC, 128], BF16, tag="wq")
            wk_sb = constp.tile([128, NHP, NC, 128], BF16, tag="wk")
            wv_sb = constp.tile([128, NHP, NC, 128], BF16, tag="wv")
            biasL = constp.tile([128, 128], BF16, tag="biasL")
            nc.sync.dma_start(biasL[:], biasLT[:])
            biasR = constp.tile([128, 128], BF16, tag="biasR")
            nc.sync.dma_start(biasR[:], biasRT[:])
            nc.sync.dma_start(wq_sb[:, 0], wqT[:, 0])
            cos_sb = constp.tile([128, S], BF16, tag="cos")
            nc.sync.dma_start(cos_sb[:], cosT[:])
            sin_sb = constp.tile([128, S], BF16, tag="sin")
            nc.sync.dma_start(sin_sb[:], sinT[:])
            nc.sync.dma_start(wk_sb[:, 0], wkT[:, 0])
            nc.sync.dma_start(wv_sb[:, 0], wvT[:, 0])

            # PE warm-up: a chain of throwaway matmuls on the (tiny,
            # first-loaded) bias constants keeps the tensor engine busy
            # through the DMA/runtime warm-up window, so the HAM clock is
            # ramped by the time the first real projection issues.  The
            # output is never read.
            for i in range(48):
                pd = psM.tile([128, 128], F32, tag="psM")
                nc.tensor.matmul(pd[:], biasL[:], biasR[:],
                                 start=True, stop=True)
            xt_sb = constp.tile([128, NC, S], BF16, tag="xt")
            for c in range(NC):
                if c % 3 == 0:
                    nc.gpsimd.dma_start(out=xt_sb[:, c, :], in_=xT[:, c, :])
                elif c % 3 == 1:
                    nc.scalar.dma_start(out=xt_sb[:, c, :], in_=xT[:, c, :])
                else:
                    nc.sync.dma_start(xt_sb[:, c, :], xT[:, c, :])
            for hp in range(1, NHP):
                nc.sync.dma_start(wq_sb[:, hp], wqT[:, hp])
                nc.sync.dma_start(wk_sb[:, hp], wkT[:, hp])
                nc.sync.dma_start(wv_sb[:, hp], wvT[:, hp])
            wo_sb = constp.tile([128, NC, SQT], BF16, tag="wo")
            nc.sync.dma_start(wo_sb[:], woT[:])
            ident = constp.tile([128, 128], BF16, tag="ident")
            make_identity(nc, ident[:])

            # persistent per-(head-pair, sq-tile) K / V / Q
            kT = [[constp.tile([128, SQT], BF16, tag=f"kT{hp}_{jj}",
                               name=f"kT{hp}_{jj}") for jj in range(NSQ)]
                  for hp in range(NHP)]
            # vaug lhsT columns per sk 128-block: [64 v, ones] per head, so
            # the PV matmuls deposit both heads' denominators on PSUM row 64
            vaug = [[constp.tile([128, 4, 130], BF16, tag=f"va{hp}_{jj}",
                                 name=f"va{hp}_{jj}") for jj in range(NSQ)]
                    for hp in range(NHP)]

            qT_store = {}

            # --- projection emitters --------------------------------------
            def rope_into(dst_ap, ps, jsl):
                # one ACT copy evacuates the projection PSUM (bank frees
                # after it); sin_sb is the PRE-SHUFFLED sin table, using
                # shuffle(x)*sin = shuffle(x*shuffle(sin)), so the gpsimd
                # multiply feeds the SBUF stream_shuffle.
                qsb = ropep.tile([128, SQT], BF16, tag="qsb")
                nc.scalar.copy(qsb[:], ps[:])
                t1 = ropep.tile([128, SQT], BF16, tag="t1")
                nc.vector.tensor_tensor(out=t1[:], in0=qsb[:],
                                        in1=cos_sb[:, jsl], op=mult)
                ts_ = ropep.tile([128, SQT], BF16, tag="ts")
                nc.vector.tensor_tensor(out=ts_[:], in0=qsb[:],
                                        in1=sin_sb[:, jsl], op=mult)
                sh = ropep.tile([128, SQT], BF16, tag="sh")
                nc.vector.stream_shuffle(sh[:], ts_[:], SWAP16)
                nc.vector.tensor_tensor(out=dst_ap, in0=t1[:], in1=sh[:],
                                        op=add)

            def emit_q(hp, j):
                jsl = bass.ts(j, SQT)
                ps = psM.tile([128, SQT], F32, tag="psM")
                for c in range(NC):
                    nc.tensor.matmul(ps[:], wq_sb[:, hp, c, :],
                                     xt_sb[:, c, jsl],
                                     start=(c == 0), stop=(c == NC - 1))
                    yield
                qt = constp.tile([128, SQT], BF16, tag=f"qT{hp}_{j}",
                                 name=f"qT{hp}_{j}")
                rope_into(qt[:], ps, jsl)
                qT_store[(hp, j)] = qt

            def emit_k(hp, j):
                jsl = bass.ts(j, SQT)
                ps = psM.tile([128, SQT], F32, tag="psM")
                for c in range(NC):
                    nc.tensor.matmul(ps[:], wk_sb[:, hp, c, :],
                                     xt_sb[:, c, jsl],
                                     start=(c == 0), stop=(c == NC - 1))
                    yield
                rope_into(kT[hp][j][:, :], ps, jsl)

            def emit_v(hp, j):
                jsl = bass.ts(j, SQT)
                ps = psM.tile([128, SQT], F32, tag="psM")
                for c in range(NC):
                    nc.tensor.matmul(ps[:], wv_sb[:, hp, c, :],
                                     xt_sb[:, c, jsl],
                                     start=(c == 0), stop=(c == NC - 1))
                    yield
                vt_sb = vtp.tile([128, SQT], BF16, tag="vt")
                nc.scalar.copy(vt_sb[:], ps[:])
                va = vaug[hp][j]
                nc.vector.memset(va[:, :, 64], 1.0)
                nc.vector.memset(va[:, :, 129], 1.0)
                for t in range(SQT // 128):
                    tp = psM.tile([128, 128], BF16, tag="psM")
                    nc.tensor.transpose(
                        tp[:], vt_sb[:, bass.ts(t, 128)], ident[:])
                    nc.vector.tensor_copy(va[:, t, 0:64], tp[:, 0:64])
                    nc.vector.tensor_copy(va[:, t, 65:129], tp[:, 64:128])
                    yield

            # --- Wo emitter (one unit per 128-row output block).  hp-major
            # matmul order matches AllGather arrival order, so only the
            # last two matmuls wait on the tile's final gather. -----------
            def emit_wo(sb, t, ag0c, ag1c):
                tsl = bass.ts(t, 128)
                ps = psM.tile([128, SQT], F32, tag="psM")
                n8 = 0
                for hp in range(NHP):
                    for g, agc in ((0, ag0c), (1, ag1c)):
                        nc.tensor.matmul(
                            ps[:], agc[:, hp, tsl],
                            wo_sb[:, NHP * g + hp, :],
                            start=(n8 == 0), stop=(n8 == 7))
                        n8 += 1
                        yield
                ost = ostp.tile([128, SQT], F32, tag="ost")
                nc.vector.tensor_copy(ost[:], ps[:])
                # scalar ring: keeps the store off the sync ring, whose
                # in-order queue stalls on gather-completion waits
                nc.scalar.dma_start(out[bass.ts(sb, 128), :], ost[:])

            # --- filler machinery: generators yielding per-matmul ---------
            projq = []            # list of [generator, steps_left, hp, j]
            woq = []
            state = {"blocks_left": 160, "werr": 0.0, "wo_hold": 0, "rr": 0}

            def _advance(q, n, rr=False):
                while n > 0 and q:
                    # round-robin between the first two generators so the
                    # two psM PSUM slots alternate and a rope tail on one
                    # does not head-block the other
                    k = 0
                    if rr and len(q) > 1:
                        state["rr"] ^= 1
                        k = state["rr"]
                    ent = q[k]
                    try:
                        next(ent[0])
                        ent[1] = max(ent[1] - 1, 0)
                        n -= 1
                    except StopIteration:
                        q.pop(k)

            def drain_proj_through(hp, j1):
                # force-complete all projections for sq tiles ordered
                # before j1 and for (hp' <= hp, j1): called inside the
                # previous segment so every dependency is ready a full
                # segment early.
                tgt = [e for e in projq
                       if ORD[e[3]] < ORD[j1] or (e[3] == j1 and e[2] <= hp)]
                while tgt:
                    for e in list(tgt):
                        try:
                            next(e[0])
                        except StopIteration:
                            tgt.remove(e)
                            if e in projq:
                                projq.remove(e)

            def pop_fillers(extra=0):
                # aggressive fixed-rate projection drip (front-loading
                # keeps the PE dense so the HAM clock stays at full rate)
                if projq:
                    _advance(projq, 5 + extra, rr=True)
                bl = max(state["blocks_left"], 1)
                if state["wo_hold"] > 0:
                    state["wo_hold"] -= 1
                elif woq:
                    state["werr"] += sum(e[1] for e in woq) / bl
                    k = int(state["werr"])
                    if k > 0:
                        state["werr"] -= k
                        _advance(woq, k)

            # --- attention for one (hp, j) --------------------------------
            def attention(hp, j):
                n = 4 * j + 4
                qt = qT_store.pop((hp, j))
                pv = psPV.tile([128, 2 * SQT], F32, tag="pv")
                qk = {}
                probs = {}

                def emit_qk(i):
                    m = i - 4 * j
                    q2 = psQK.tile([128, 2 * SQT], F32, tag="qk")
                    diag = m >= 0
                    w = SQT - 128 * max(m, 0)
                    for h in range(2):
                        nc.tensor.matmul(
                            q2[:, SQT * h:SQT * h + w],
                            kT[hp][i // 4][64 * h:64 * h + 64,
                                           bass.ts(i % 4, 128)],
                            qt[64 * h:64 * h + 64, SQT - w:SQT],
                            start=True, stop=not diag,
                            skip_group_check=diag)
                    if diag:
                        # fold the causal triangle into PSUM: adds -B to the
                        # masked half of the leading 128-wide strip, so exp
                        # yields ~0 there and no mask multiply is needed
                        for h in range(2):
                            nc.tensor.matmul(
                                q2[:, SQT * h:SQT * h + 128],
                                biasL[:], biasR[:],
                                start=False, stop=True,
                                skip_group_check=True)
                    qk[i] = q2

                def emit_exp(i):
                    m = i - 4 * j
                    pr = probsp.tile([128, 2 * SQT], BF16, tag="pr")
                    if m <= 0:
                        nc.scalar.activation(pr[:], qk[i][:], Exp, scale=0.125)
                    else:
                        w_tot = 2 * SQT - 128 * m
                        nc.scalar.activation(pr[:, 0:w_tot], qk[i][:, 0:w_tot],
                                             Exp, scale=0.125)
                    del qk[i]
                    probs[i] = pr

                def emit_pv(i):
                    m = i - 4 * j
                    first = (i == 0)
                    last = (i == n - 1)
                    pr = probs.pop(i)
                    for h in range(2):
                        vsl = vaug[hp][i // 4][:, i % 4, 65 * h:65 * h + 65]
                        if m <= 0:
                            nc.tensor.matmul(
                                pv[0:65, SQT * h:SQT * h + SQT], vsl,
                                pr[:, bass.ts(h, SQT)],
                                start=first, stop=last)
                        else:
                            w = SQT - 128 * m
                            nc.tensor.matmul(
                                pv[0:65, SQT * h + 128 * m:SQT * h + SQT],
                                vsl, pr[:, SQT * h:SQT * h + w],
                                start=False, stop=last,
                                skip_group_check=True)

                emit_qk(0)
                pop_fillers(extra=1)
                emit_qk(1)
                pop_fillers(extra=1)
                for i in range(n):
                    emit_exp(i)
                    state["blocks_left"] -= 1
                    pop_fillers()
                    if i + 2 < n:
                        emit_qk(i + 2)
                    emit_pv(i)

                return pv

            def norm_segment(hp, pv):
                # denominator row copy + raw bf16 copy of the attention
                # rows: the PV banks free after these two DVE ops.  The
                # reciprocal, broadcasts and normalization multiplies trail
                # off the critical path.
                den = normp.tile([1, 2 * SQT], F32, tag="den")
                nc.vector.tensor_copy(den[:], pv[64:65, :])
                pvs = normp.tile([64, 2 * SQT], BF16, tag="pvs")
                nc.vector.tensor_copy(pvs[:], pv[0:64, :])
                rec = normp.tile([1, 2 * SQT], F32, tag="rec")
                nc.vector.reciprocal_approx_fast(out=rec[:], in_=den[:])
                reca = normp.tile([64, SQT], F32, tag="reca")
                nc.gpsimd.partition_broadcast(reca[:], rec[0:1, 0:SQT],
                                              channels=64)
                recb = normp.tile([64, SQT], F32, tag="recb")
                nc.gpsimd.partition_broadcast(recb[:], rec[0:1, SQT:2 * SQT],
                                              channels=64)
                nc.vector.tensor_tensor(
                    out=attnT_j[0:64, hp, :], in0=pvs[:, 0:SQT],
                    in1=reca[:], op=mult)
                nc.vector.tensor_tensor(
                    out=attnT_j[64:128, hp, :], in0=pvs[:, SQT:2 * SQT],
                    in1=recb[:], op=mult)

            def run_all(gen):
                for _ in gen:
                    pass

            # --- prologue: projections for j = 0 --------------------------
            for hp in range(NHP):
                run_all(emit_q(hp, 0))
                run_all(emit_k(hp, 0))
                run_all(emit_v(hp, 0))

            for j1 in TILE_ORDER[1:]:
                for hp in range(NHP):
                    projq.append([emit_q(hp, j1), 9, hp, j1])
                    projq.append([emit_k(hp, j1), 9, hp, j1])
                    projq.append([emit_v(hp, j1), 13, hp, j1])

            # --- main loop ------------------------------------------------
            for oi, j in enumerate(TILE_ORDER):
                nxt = TILE_ORDER[oi + 1] if oi + 1 < NSQ else None

                state["blocks_left"] = NHP * (4 * j + 4)
                attnT_j = attnp.tile([128, NHP, SQT], BF16, tag="attnT")
                ag0c = agp.tile([128, NHP, SQT], BF16, tag="ag0c")
                ag1c = agp.tile([128, NHP, SQT], BF16, tag="ag1c")
                last_tile = oi == NSQ - 1
                for hp in range(NHP):
                    pv = attention(hp, j)
                    if nxt is not None:
                        drain_proj_through(hp, nxt)
                    norm_segment(hp, pv)
                    # per-head-pair AllGather so the pair's Wo work can
                    # start before the whole tile finishes.  On the last
                    # tile, gather two head-pairs at a time instead: its Wo
                    # runs only in the epilogue, and halving the CC ops gets
                    # the final gather's data out ~15us sooner.
                    # ag_in staged via the gpsimd ring: the sync ring's
                    # in-order queue would make it wait behind the PREVIOUS
                    # gather's readback, serializing the collectives.
                    # last tile: hp0+hp1 go out as one paired gather (its Wo
                    # runs only in the epilogue), hp2 and hp3 go solo so
                    # hp2's chunks land during hp3's attention and only
                    # hp3's small solo gather is exposed at the tail
                    if not (last_tile and hp == 0):
                        nhp = 2 if (last_tile and hp == 1) else 1
                        h0 = hp - nhp + 1
                        ag_in = dramp.tile([128, nhp, SQT], BF16,
                                           tag=f"ag_in{nhp}")
                        nc.gpsimd.dma_start(
                            out=ag_in[:], in_=attnT_j[:, h0:hp + 1, :])
                        ag_out = dramp.tile([2, 128, nhp, SQT], BF16,
                                            tag=f"ag_out{nhp}")
                        nc.gpsimd.collective_compute(
                            "AllGather", mybir.AluOpType.bypass,
                            ins=[ag_in[:].opt()], outs=[ag_out[:].opt()],
                            replica_groups=groups)
                        nc.sync.dma_start(ag0c[:, h0:hp + 1, :], ag_out[0])
                        nc.sync.dma_start(ag1c[:, h0:hp + 1, :], ag_out[1])

                # queue Wo for this tile; it drains during the next tile's
                # attention (filling the exp-latency bubbles), the final
                # tile's in the epilogue
                for t in range(SQT // 128):
                    sb = (SQT // 128) * j + t
                    woq.append([emit_wo(sb, t, ag0c, ag1c), 9, 0, j])
                state["wo_hold"] = 4

            while projq:
                _advance(projq, 1)
            while woq:
                _advance(woq, 1)

    nc.compile()
    return nc


# ---------------------------------------------------------------------------
# Host-side sharding / unsharding
# ---------------------------------------------------------------------------

def _host_inputs(x, Wq, Wk, Wv, Wo, token_positions, n_cores, S):
    import ml_dtypes
    bf16 = ml_dtypes.bfloat16
    D = D_MODEL
    NC = D // 128
    NHP = 4

    # rope tables.  Partition layout within each head (64 partitions):
    # [e0..e15, o0..o15, e16..e31, o16..o31] -- the rotation partner sits
    # 16 partitions away inside the same 32-group, so the kernel's
    # stream_shuffle (a per-32-group lane shuffle) can realize the swap.
    pos = np.asarray(token_positions).astype(np.float32)  # (S,)
    i32 = np.arange(32, dtype=np.float32)
    inv_freq = ROPE_THETA ** (-i32 / 32.0)
    ang = pos[None, :] * inv_freq[:, None]              # (32, S)
    p = np.arange(128)
    pp = p % 64
    g, o = pp // 32, pp % 32
    freq_idx = 16 * g + (o % 16)                        # (128,)
    sign = np.where(o % 32 < 16, -1.0, 1.0)             # even slots: -sin
    cosT = np.cos(ang[freq_idx, :]).astype(bf16)        # (128, S)
    sinT = (sign[:, None] * np.sin(ang[freq_idx, :])).astype(bf16)
    # pre-shuffled sin table: the kernel computes shuffle(x * shuffle(sin))
    # instead of shuffle(x) * sin, so the stream_shuffle input is SBUF
    swap = (np.arange(128) // 32) * 32 + (np.arange(128) % 32 + 16) % 32
    sinT = np.ascontiguousarray(sinT[swap, :])

    # triangle-bias matmul constants: (biasL.T @ biasR)[p, f] = -B iff p > f
    # (the causally-masked half of a diagonal 128-strip); exp then gives ~0
    biasL = (np.arange(128)[None, :] > np.arange(128)[:, None]).astype(bf16)
    biasR = (-400.0 * np.eye(128, dtype=np.float32)).astype(bf16)

    # de-interleaving row permutation for q/k (see rope table comment)
    def qk_rows(grp):
        rows = []
        for h in range(8 * grp, 8 * grp + 8):
            rows += [h * DH + 2 * i for i in range(16)]
            rows += [h * DH + 2 * i + 1 for i in range(16)]
            rows += [h * DH + 2 * i for i in range(16, 32)]
            rows += [h * DH + 2 * i + 1 for i in range(16, 32)]
        return rows

    def wqk_layout(W, grp):
        # (D, 512) -> [128, NHP, NC, 128]
        t = W[qk_rows(grp), :].T.astype(bf16)
        return np.ascontiguousarray(
            t.reshape(NC, 128, NHP, 128).transpose(1, 2, 0, 3))

    def wv_layout(W, grp):
        t = W[512 * grp:512 * grp + 512, :].T.astype(bf16)
        return np.ascontiguousarray(
            t.reshape(NC, 128, NHP, 128).transpose(1, 2, 0, 3))

    def wo_layout(W, grp):
        t = W.T[:, 512 * grp:512 * grp + 512].astype(bf16)  # (D, 512)
        return np.ascontiguousarray(t.reshape(NC, 128, SQT).transpose(1, 0, 2))

    in_maps = []
    for c in range(n_cores):
        b, grp = c // 2, c % 2
        xb = np.ascontiguousarray(x[b].T).astype(bf16)  # (D, S)
        in_maps.append({
            "xT": np.ascontiguousarray(
                xb.reshape(NC, 128, S).transpose(1, 0, 2)),
            "wqT": wqk_layout(Wq, grp),
            "wkT": wqk_layout(Wk, grp),
            "wvT": wv_layout(Wv, grp),
            "woT": wo_layout(Wo, grp),
            "cosT": cosT,
            "sinT": sinT,
            "biasLT": biasL,
            "biasRT": biasR,
        })
    return in_maps


def _assemble(results, n_cores, S):
    B = n_cores // 2
    full = np.empty((B, S, D_MODEL), dtype=np.float32)
    for c in range(n_cores):
        b, grp = c // 2, c % 2
        full[b, :, 512 * grp:512 * grp + 512] = results[c]["out"]
    return full


# ---------------------------------------------------------------------------
# Entry point
# ---------------------------------------------------------------------------

_NC_CACHE = {}


def _get_nc(n_cores, S):
    key = (n_cores, S)
    if key not in _NC_CACHE:
        _NC_CACHE[key] = build_kernel(n_cores, S)
    return _NC_CACHE[key]


def kernel(x, Wq, Wk, Wv, Wo, token_positions, _trace=False, _tmpdir=None):
    from concourse.bass_utils import run_bass_kernel_spmd

    x = np.asarray(x)
    B, S, D = x.shape
    n_cores = 2 * B
    nc = _get_nc(n_cores, S)
    in_maps = _host_inputs(np.asarray(x), np.asarray(Wq), np.asarray(Wk),
                           np.asarray(Wv), np.asarray(Wo),
                           np.asarray(token_positions), n_cores, S)
    res = run_bass_kernel_spmd(nc, in_maps, core_ids=list(range(n_cores)),
                               trace=_trace, tmpdir=_tmpdir)
    out = _assemble(res.results, n_cores, S)
    if _trace:
        return out, res
    return out
